# revision 41
# baseline (speedup 1.0000x reference)
"""Trainium2 Bass kernel for nn_CapLayer_90056874263182.

Math note: the reference initializes routing logits b0 = zeros, so the
softmax over the 10 output caps starts uniform; s, v and delta_b are then
identical across caps, so the logits stay equal across caps through every
routing iteration and the softmax stays uniform forever.  The routing loop
therefore collapses exactly to

    v[b, o, :] = squash((1/10) * sum_i pred[b, i, :])   for every o

and  sum_i pred[b,i,:] = sum_{c,p} x[b,c,p] * W[c//8,:,p%8] + 144*sum_s Wb[s,:]
(the row-major reshape maps in_dim to p%8).

Kernel per core (64 batches), fp8 data path (tolerance is 2e-2; fp8_e3m4
x plus 16x-prescaled fp8 weights measure 6.9e-3 end to end on the exact
harness inputs, fp16 would be 1.4e-4):
  - host packs the x shard as [128 part, 288 t', 64 b] float8_e3m4 where
    the flat contraction index f = c*144 + p is split as f = part*288+t';
    partition `part` always uses weight group part//4 and in_dim t'%8.
    e3m4 has 4 mantissa bits and range +-15.5: x ~ N(0,1) fits directly,
    W (~0.1 scale) is prescaled by 16 to escape the subnormal range and
    the 16x is folded back inside the squash scales.
  - the whole contraction runs on the PE: 288 accumulating fp8 matmuls
    lhsT = x[:, t', :] [128, 64], rhs = W column block (t'%8) [128, 16]
    into PSUM S' = 16*S [64, 17] (f32 accumulate).  A K=1 f32 matmul
    OPENS the group with the bias row (16*brow, exact) plus a constant
    pad column whose square makes the squash denominator's "+1" for free
  - DMA is the roofline: 2.36 MB fp8 per core streamed as 9 tapered
    column tiles.  Few tiles on purpose: the (global, serialized) HWDGE
    descriptor generator costs ~625 ns per DMA and must stay ahead of a
    6.6 us stream.  The weight columns ride inside tile 0 and the f32
    consts row rides SWDGE (GPSIMD desc-gen), so neither costs an HWDGE
    slot.  chunk >= 8 keeps innermost segments >= 512 B (full DMA rate).
  - squash on ACT/DVE with the 1/160 scale folded in, out [64, 16] f32;
    host replicates the 10 identical caps

Timeline (TimelineSim cost model, per core): ~1.35 us to first DMA byte
(preamble + HWDGE + DGE-to-DMA delay), 6.6 us gapless fp8 stream, then a
~4.4 us latency tail that is almost entirely framework constants (900 ns
DMA-sem propagation x2, 1.3 us out-DMA issue, ~0.3 us exit barrier).
33974 ns baseline -> 12335 ns.
"""

import numpy as np

BS = 512          # full batch
NC = 8            # cores
B = BS // NC      # batches per core
CH = 256          # channels
HW = 144          # h*w
F = CH * HW       # flat contraction length per batch = 36864
TP = F // 128     # t' chunks per partition = 288
I8 = 8            # in_dim (= p % 8 bucket)
D = 16            # out_dim
NO = 10           # num output caps
WS = 16.0         # fp8 weight prescale (1/WS folded into squash scales)

# DMA column tiles in t'-chunk units (each chunk = 64 batches x 128 parts
# x 1 B = 8 KB, ~22.8 ns of stream).  Big-to-small taper: only the LAST
# tile's matmuls sit on the critical path after the final DMA semaphore,
# and the second-to-last must leave the PE enough slack to drain first.
CHUNKS = [56, 48, 44, 36, 32, 28, 20, 16, 8]
assert sum(CHUNKS) == TP
assert all(c >= 8 for c in CHUNKS)
WRC = 2           # wr8's 128 weight columns = 2 chunk-equivalents in tile 0

# Skip the four const-AP GPSIMD memsets Bass.__init__ emits before the
# entry all-engine barrier: they hold the barrier (and with it the first
# x DMA) back by ~400 ns, and nothing in this kernel reads a const AP
# (all activation biases / DVE scalars are explicit SBUF tiles).
SKIP_CONST_APS = True


def _build_nc():
    from contextlib import ExitStack

    import concourse.bass as bass
    import concourse.mybir as mybir
    import concourse.tile as tile
    from concourse import bacc

    f32 = mybir.dt.float32
    f8 = mybir.dt.float8e3
    AF = mybir.ActivationFunctionType

    if SKIP_CONST_APS:
        cls = bass.BassEitherVectorEngine
        orig_memset = cls.memset

        def _memset_skip_const(self, ap, constant):
            name = str(getattr(getattr(ap, "tensor", None), "name", "") or "")
            if name.startswith("const-"):
                return None
            return orig_memset(self, ap, constant)

        # With the const memsets gone the init-time all-engine barrier has
        # nothing left to order (Tile manages every later dependency with
        # explicit semaphores), so skip it too: it otherwise holds the
        # first x DMA back by ~250 ns.
        orig_barrier = bass.Bass.all_engine_barrier
        cls.memset = _memset_skip_const
        bass.Bass.all_engine_barrier = lambda self, **kw: None
        try:
            nc = bacc.Bacc()
        finally:
            cls.memset = orig_memset
            bass.Bass.all_engine_barrier = orig_barrier
    else:
        nc = bacc.Bacc()

    # x stream plus, appended to tile 0's columns, the fp8 weight col
    # blocks wr8[p, i*16+d] = WS * W[p//4, d, i] (one DMA, full rate)
    xt_d = nc.dram_tensor("xt", [128, (TP + WRC) * B], f8, kind="ExternalInput")
    # f32 row: [0:64] ones (K=1 lhsT), [64:80] = WS * 144 * sum_s Wb[s, :],
    # [80] = 160 (PSUM pad column making the Square accumulator = 1 + nsq)
    wb_d = nc.dram_tensor("wb", [1, B + D + 1], f32, kind="ExternalInput")
    v_d = nc.dram_tensor("v", [B, D], f32, kind="ExternalOutput")

    # Teardown diet: TileContext exit emits drain+waits -> barrier -> sem
    # clears -> barrier.  The clears only matter if the NEFF re-executes
    # with stale semaphores; every kernel() call compiles and runs a fresh
    # NEFF exactly once, so keep the drain waits and ONE final barrier and
    # skip the clear round (~0.3 us of serialized exit latency).
    from concourse.vector_clock import ScopedClock

    def _lean_drain_and_barrier(self, tick_clock, wait_clock):
        drain_inst = self.nc.sync.drain()
        wait_clock.add_sem_waits(
            drain_inst.ins, ScopedClock({None: tick_clock.global_clock})
        )
        self.nc.all_engine_barrier()
        popped = self.nc._tile_sem_poison_stack.pop()
        assert popped is self._sem_poison
        assert self.sems is not None
        for h in self.sems.allocated().values():
            self.nc.release_semaphore(h)

    tile.TileContext._drain_and_barrier = _lean_drain_and_barrier

    with tile.TileContext(nc) as tc, ExitStack() as ctx:
        consts = ctx.enter_context(tc.tile_pool(name="consts", bufs=1))
        xpool = ctx.enter_context(tc.tile_pool(name="xin", bufs=len(CHUNKS)))
        small = ctx.enter_context(tc.tile_pool(name="small", bufs=1))
        psum = ctx.enter_context(tc.tile_pool(name="psum", bufs=2, space="PSUM"))

        # x tiles stream on HWDGE (SP engine); tile 0 carries the fp8
        # weight columns appended after its x chunks, and the tiny f32
        # consts row rides SWDGE via GPSIMD — neither costs an extra HWDGE
        # slot (the global HWDGE takes ~625 ns per DMA and must stay ahead
        # of a 6.6 us stream).
        xts = []
        doff = 0
        for idx, c in enumerate(CHUNKS):
            cc = c + (WRC if idx == 0 else 0)
            xt = xpool.tile([128, cc * B], f8, tag=f"xt{idx}", bufs=1)
            nc.sync.dma_start(xt[:, :], xt_d[:, doff * B : (doff + cc) * B])
            xts.append(xt)
            doff += cc
        wr8 = xts[0][:, CHUNKS[0] * B : (CHUNKS[0] + WRC) * B]
        # f32 consts row rides SWDGE (GPSIMD desc-gen): lands mid-stream,
        # well before tile 0's matmuls need the group-opening bias matmul
        wb = consts.tile([1, B + D + 1], f32)
        nc.gpsimd.dma_start(wb[:, :], wb_d[:, :])

        onesb = wb[0:1, 0:B]
        bres = wb[0:1, B : B + D + 1]
        # explicit zero-bias AP: a float bias would be lowered to a const AP
        # (skipped above), so activations get this tile instead
        zb = consts.tile([B, 1], f32)
        nc.vector.memset(zb[:, :], 0.0)
        # Sqrt bias -0.01/256: rt = sqrt((den - 1) * 0.01/256) = 0.1*|m|/16
        nb = consts.tile([B, 1], f32)
        nc.vector.memset(nb[:, :], -0.01 / (WS * WS))
        # DVE warm-up + early ACT Sqrt: pins the sqrt_and_others table
        # (holds Sqrt, Square, Copy) at t~0 instead of in the tail.
        scr = consts.tile([1, 1], f32)
        nc.vector.memset(scr[:, :], 1.0)
        scr2 = consts.tile([1, 1], f32)
        nc.scalar.activation(scr2[:, :], scr[:, :], AF.Sqrt, bias=zb[0:1, :])

        # S'[b, d] = WS*brow[d] + WS * sum_{p, t'} x[p,t',b] * W[p//4, d, t'%8]
        # The K=1 f32 bias matmul OPENS the [64, 17] group (start=True zeroes
        # the pad column too) — its wb const lands mid-stream via SWDGE well
        # before tile 0's matmuls; col 16 gets the constant 160 so the Square
        # accumulator below directly yields den = 1 + nsq (no DVE add).
        ps = psum.tile([B, D + 1], f32)
        nc.tensor.matmul(
            ps[:, :], onesb, bres, start=True, stop=False, skip_group_check=True
        )
        off = 0
        for t, c in enumerate(CHUNKS):
            xv = xts[t][:, : c * B].rearrange("p (c b) -> p c b", c=c)
            for j in range(c):
                i = (off + j) % I8
                nc.tensor.matmul(
                    ps[:, 0:D],
                    xv[:, j, :],
                    wr8[:, i * D : (i + 1) * D],
                    start=False,
                    stop=(off + j == TP - 1),
                    skip_group_check=True,
                )
            off += c

        # squash with m = S/10 = S'/(10*WS) folded into the scales:
        #   den = 1 + |m|^2 = sum_d (S'_pad/160)^2   (pad col -> the 1)
        #   rt = 0.1*|m|/16 = sqrt(den*0.01/256 - 0.01/256),  rec = 1/den
        #   v_row = S' * rt * rec
        # DVE stages S' into SBUF in parallel with the ACT Square: the
        # final multiply then pays SBUF access latency (58 cyc) instead of
        # PSUM (120 cyc) on the critical path.
        psb = small.tile([B, D], f32)
        nc.vector.tensor_copy(psb[:, :], ps[:, 0:D])
        sq = psum.tile([B, D + 1], f32)
        den = small.tile([B, 1], f32)
        nc.scalar.activation(
            sq[:, :], ps[:, :], AF.Square, bias=zb[:, :], scale=1.0 / (10.0 * WS),
            accum_out=den[:, :],
        )
        rt = small.tile([B, 1], f32)
        nc.scalar.activation(
            rt[:, :], den[:, :], AF.Sqrt, bias=nb[:, :], scale=0.01 / (WS * WS)
        )
        # rec on DVE overlaps the ACT Sqrt
        rec = small.tile([B, 1], f32)
        nc.vector.reciprocal(rec[:, :], den[:, :])

        # v_row = (S' * rt) * rec in one dual-scalar DVE op
        vrow = small.tile([B, D], f32)
        nc.vector.tensor_scalar(
            vrow[:, :],
            psb[:, :],
            rt[:, :],
            rec[:, :],
            op0=mybir.AluOpType.mult,
            op1=mybir.AluOpType.mult,
        )
        nc.sync.dma_start(v_d[:, :], vrow[:, :])

    nc.finalize()

    # Teardown reorder: Tile emits the SP exit waits as five 2-wait
    # EventSemaphores draining every DMA lane + engine sem.  The out-DMA's
    # completion wait lands in the FIRST of them, so the other four decode
    # (~50 ns each) after the final DMA sem fires — pure added tail.  Move
    # that wait into the LAST instruction of the run (the wait set gating
    # the drains is unchanged, waits commute).
    insts = [i for blk in nc.m.functions[0].blocks for i in blk.instructions]

    # Disarm EventSemaphore waits that only reference their OWN engine's
    # tick semaphore: the engine executes its queue in order, so the
    # dependency already holds; the wait just parks the sequencer and
    # pushes the consumer's decode past the producing op's completion.
    eng_sem = {
        "EngineType.DVE": "DVE_",
        "EngineType.Activation": "Activation_",
        "EngineType.PE": "PE_",
        "EngineType.Pool": "Pool_",
        "EngineType.SP": "SP_",
    }
    for x in insts:
        if type(x).__name__ != "InstEventSemaphore" or x.sync_info is None:
            continue
        pref = eng_sem.get(str(x.engine))
        if not pref:
            continue
        for w in x.sync_info.on_wait:
            nm = w.ant_name or ""
            if nm.startswith(pref) and nm[len(pref):].isdigit():
                w.wait_value = 0

    dmas = [x for x in insts if type(x).__name__ == "InstDMACopy"]
    out_sem = dmas[-1].sync_info.on_update[0].id
    run = [
        x
        for x in insts
        if type(x).__name__ == "InstEventSemaphore"
        and str(x.engine) == "EngineType.SP"
        and x.sync_info is not None
        and len(x.sync_info.on_wait) == 2
        and any("DMAHW" in (w.ant_name or "") for w in x.sync_info.on_wait)
    ]
    if len(run) >= 2:
        src = None
        for x in run[:-1]:
            for w in x.sync_info.on_wait:
                if w.id == out_sem:
                    src = w
        if src is not None:
            dst = next(
                w for w in run[-1].sync_info.on_wait if w.id != out_sem
            )
            for fld in ("id", "ant_name", "wait_value"):
                a, b = getattr(src, fld), getattr(dst, fld)
                try:
                    setattr(src, fld, b)
                    setattr(dst, fld, a)
                except Exception:
                    pass
    return nc


def _host_inputs(x, W, Wb):
    import ml_dtypes

    f8 = ml_dtypes.float8_e3m4
    x = np.ascontiguousarray(np.asarray(x, dtype=np.float32)).reshape(BS, F)
    W = np.asarray(W, dtype=np.float32)
    Wb = np.asarray(Wb, dtype=np.float32)

    # wr8[p, i*16 + d] = WS * W[p//4, d, i]
    s_of_p = np.arange(128) // 4
    wrj = np.empty((128, I8, D), dtype=np.float32)
    for i in range(I8):
        wrj[:, i, :] = W[s_of_p, :, i]
    wr8 = (WS * wrj.reshape(128, I8 * D)).astype(f8)

    # x shards: [128 part, TP t', B b] fp8, flat f = part*TP + t'
    # (f = c*144 + p; partition part spans f in [part*288, part*288+288),
    #  always inside weight group part//4 since 1152 = 4*288), with the
    # wr8 columns spliced in right after tile 0's chunks
    c0 = CHUNKS[0] * B
    xs = []
    for k in range(NC):
        xk = np.ascontiguousarray(x[k * B : (k + 1) * B].T.astype(f8)).reshape(
            128, TP * B
        )
        xs.append(np.concatenate([xk[:, :c0], wr8, xk[:, c0:]], axis=1))

    # f32 consts row: ones for the K=1 bias matmul, WS*144*sum_s Wb, and
    # the 160 pad value whose square (after the 1/160 Square scale) is the
    # "+1" of the squash denominator
    wb = np.empty((1, B + D + 1), dtype=np.float32)
    wb[0, :B] = 1.0
    wb[0, B : B + D] = WS * HW * Wb.sum(axis=0)
    wb[0, B + D] = 10.0 * WS
    return xs, wb


def _run(x, W, Wb, trace=False):
    from concourse.bass_utils import run_bass_kernel_spmd

    xs, wb = _host_inputs(x, W, Wb)
    nc = _build_nc()
    in_maps = [{"xt": xs[k], "wb": wb} for k in range(NC)]
    res = run_bass_kernel_spmd(nc, in_maps, list(range(NC)), trace=trace)
    rows = np.concatenate([res.results[k]["v"] for k in range(NC)], axis=0)
    # unshard: replicate the (identical) caps into the full [BS, NO, D] shape
    out = np.ascontiguousarray(
        np.broadcast_to(rows.reshape(BS, 1, D), (BS, NO, D)), dtype=np.float32
    )
    return out, res


def _numpy_fallback(x, W, Wb, b0):
    """Generic routing on the host — only used if b0 is ever nonzero
    (the spec fills b0 with zeros, which collapses the routing; see top)."""
    x = np.asarray(x, np.float32)
    W = np.asarray(W, np.float32)
    Wb = np.asarray(Wb, np.float32)
    b0 = np.asarray(b0, np.float32)
    u = x.reshape(BS, 32, HW, I8)
    pred = np.einsum("bsni,soi->bsno", u, W) + Wb[None, :, None, :]
    pred = pred.reshape(BS, 32 * HW, D)
    b = np.broadcast_to(b0, (BS,) + b0.shape).copy()
    v = None
    for _ in range(3):
        e = np.exp(b - b.max(axis=1, keepdims=True))
        c = e / e.sum(axis=1, keepdims=True)
        s = np.einsum("boi,bid->bod", c, pred)
        nrm = np.linalg.norm(s, axis=2)
        coeff = (nrm * nrm / (1.0 + nrm * nrm)) / nrm
        v = s * coeff[:, :, None]
        b = b + np.einsum("bid,bod->boi", pred, v)
    return v.astype(np.float32)


def kernel(x, W, Wb, b0=None, **_ignored):
    if b0 is not None and np.any(np.asarray(b0)):
        return _numpy_fallback(x, W, Wb, b0)
    try:
        out, _ = _run(x, W, Wb, trace=False)
    except Exception:
        # one retry: the axon-tunneled device occasionally reports a
        # transient NRT_EXEC_UNIT_UNRECOVERABLE on first touch
        out, _ = _run(x, W, Wb, trace=False)
    return out


def kernel_traced(x, W, Wb, b0=None):
    """Like kernel() but also returns the BassKernelResults (exec_time_ns)."""
    return _run(x, W, Wb, trace=True)


# revision 43
# speedup vs baseline: 1.0196x; 1.0196x over previous
"""Trainium2 Bass kernel for nn_CapLayer_90056874263182.

Math note: the reference initializes routing logits b0 = zeros, so the
softmax over the 10 output caps starts uniform; s, v and delta_b are then
identical across caps, so the logits stay equal across caps through every
routing iteration and the softmax stays uniform forever.  The routing loop
therefore collapses exactly to

    v[b, o, :] = squash((1/10) * sum_i pred[b, i, :])   for every o

and  sum_i pred[b,i,:] = sum_{c,p} x[b,c,p] * W[c//8,:,p%8] + 144*sum_s Wb[s,:]
(the row-major reshape maps in_dim to p%8).

Kernel per core (64 batches), fp8 data path (tolerance is 2e-2; fp8_e3m4
x plus 16x-prescaled fp8 weights measure 6.9e-3 end to end on the exact
harness inputs, fp16 would be 1.4e-4):
  - host packs the x shard as [128 part, 288 t', 64 b] float8_e3m4 where
    the flat contraction index f = c*144 + p is split as f = part*288+t';
    partition `part` always uses weight group part//4 and in_dim t'%8.
    e3m4 has 4 mantissa bits and range +-15.5: x ~ N(0,1) fits directly,
    W (~0.1 scale) is prescaled by 16 to escape the subnormal range and
    the 16x is folded back inside the squash scales.
  - the whole contraction runs on the PE: 288 accumulating fp8 matmuls
    lhsT = x[:, t', :] [128, 64], rhs = W column block (t'%8) [128, 16]
    into PSUM S' = 16*S [64, 17] (f32 accumulate).  A K=1 f32 matmul
    OPENS the group with the bias row (16*brow, exact) plus a constant
    pad column whose square makes the squash denominator's "+1" for free
  - DMA is the roofline: 2.36 MB fp8 per core streamed as 9 tapered
    column tiles.  Few tiles on purpose: the (global, serialized) HWDGE
    descriptor generator costs ~625 ns per DMA and must stay ahead of a
    6.6 us stream.  The weight columns ride inside tile 0 and the f32
    consts row rides SWDGE (GPSIMD desc-gen), so neither costs an HWDGE
    slot.  chunk >= 8 keeps innermost segments >= 512 B (full DMA rate).
  - squash on ACT/DVE with the 1/160 scale folded in, out [64, 16] f32;
    host replicates the 10 identical caps

Timeline (TimelineSim cost model, per core): ~1.35 us to first DMA byte
(preamble + HWDGE + DGE-to-DMA delay), 6.6 us gapless fp8 stream, then a
~4.4 us latency tail that is almost entirely framework constants (900 ns
DMA-sem propagation x2, 1.3 us out-DMA issue, ~0.3 us exit barrier).
33974 ns baseline -> 12335 ns.
"""

import numpy as np

BS = 512          # full batch
NC = 8            # cores
B = BS // NC      # batches per core
CH = 256          # channels
HW = 144          # h*w
F = CH * HW       # flat contraction length per batch = 36864
TP = F // 128     # t' chunks per partition = 288
I8 = 8            # in_dim (= p % 8 bucket)
D = 16            # out_dim
NO = 10           # num output caps
WS = 16.0         # fp8 weight prescale (1/WS folded into squash scales)

# DMA column tiles in t'-chunk units (each chunk = 64 batches x 128 parts
# x 1 B = 8 KB, ~22.8 ns of stream).  Big-to-small taper: only the LAST
# tile's matmuls sit on the critical path after the final DMA semaphore,
# and the second-to-last must leave the PE enough slack to drain first.
CHUNKS = [56, 48, 44, 36, 32, 28, 20, 16, 8]
assert sum(CHUNKS) == TP
assert all(c >= 8 for c in CHUNKS)
WRC = 2           # wr8's 128 weight columns = 2 chunk-equivalents in tile 0

# Skip the four const-AP GPSIMD memsets Bass.__init__ emits before the
# entry all-engine barrier: they hold the barrier (and with it the first
# x DMA) back by ~400 ns, and nothing in this kernel reads a const AP
# (all activation biases / DVE scalars are explicit SBUF tiles).
SKIP_CONST_APS = True


def _build_nc():
    from contextlib import ExitStack

    import concourse.bass as bass
    import concourse.mybir as mybir
    import concourse.tile as tile
    from concourse import bacc

    f32 = mybir.dt.float32
    f8 = mybir.dt.float8e3
    AF = mybir.ActivationFunctionType

    if SKIP_CONST_APS:
        cls = bass.BassEitherVectorEngine
        orig_memset = cls.memset

        def _memset_skip_const(self, ap, constant):
            name = str(getattr(getattr(ap, "tensor", None), "name", "") or "")
            if name.startswith("const-"):
                return None
            return orig_memset(self, ap, constant)

        # With the const memsets gone the init-time all-engine barrier has
        # nothing left to order (Tile manages every later dependency with
        # explicit semaphores), so skip it too: it otherwise holds the
        # first x DMA back by ~250 ns.
        orig_barrier = bass.Bass.all_engine_barrier
        cls.memset = _memset_skip_const
        bass.Bass.all_engine_barrier = lambda self, **kw: None
        try:
            nc = bacc.Bacc()
        finally:
            cls.memset = orig_memset
            bass.Bass.all_engine_barrier = orig_barrier
    else:
        nc = bacc.Bacc()

    # x stream plus, appended to tile 0's columns, the fp8 weight col
    # blocks wr8[p, i*16+d] = WS * W[p//4, d, i] (one DMA, full rate)
    xt_d = nc.dram_tensor("xt", [128, (TP + WRC) * B], f8, kind="ExternalInput")
    # f32 row: [0:64] ones (K=1 lhsT), [64:80] = WS * 144 * sum_s Wb[s, :],
    # [80] = 160 (PSUM pad column making the Square accumulator = 1 + nsq)
    wb_d = nc.dram_tensor("wb", [1, B + D + 1], f32, kind="ExternalInput")
    v_d = nc.dram_tensor("v", [B, D], f32, kind="ExternalOutput")

    # Teardown diet: TileContext exit emits drain+waits -> barrier -> sem
    # clears -> barrier.  The clears only matter if the NEFF re-executes
    # with stale semaphores; every kernel() call compiles and runs a fresh
    # NEFF exactly once, so keep the drain waits and ONE final barrier and
    # skip the clear round (~0.3 us of serialized exit latency).
    from concourse.vector_clock import ScopedClock

    def _lean_drain_and_barrier(self, tick_clock, wait_clock):
        drain_inst = self.nc.sync.drain()
        wait_clock.add_sem_waits(
            drain_inst.ins, ScopedClock({None: tick_clock.global_clock})
        )
        self.nc.all_engine_barrier()
        popped = self.nc._tile_sem_poison_stack.pop()
        assert popped is self._sem_poison
        assert self.sems is not None
        for h in self.sems.allocated().values():
            self.nc.release_semaphore(h)

    tile.TileContext._drain_and_barrier = _lean_drain_and_barrier

    with tile.TileContext(nc) as tc, ExitStack() as ctx:
        consts = ctx.enter_context(tc.tile_pool(name="consts", bufs=1))
        xpool = ctx.enter_context(tc.tile_pool(name="xin", bufs=len(CHUNKS)))
        small = ctx.enter_context(tc.tile_pool(name="small", bufs=1))
        psum = ctx.enter_context(tc.tile_pool(name="psum", bufs=2, space="PSUM"))

        # x tiles stream on HWDGE (SP engine); tile 0 carries the fp8
        # weight columns appended after its x chunks, and the tiny f32
        # consts row rides SWDGE via GPSIMD — neither costs an extra HWDGE
        # slot (the global HWDGE takes ~625 ns per DMA and must stay ahead
        # of a 6.6 us stream).
        xts = []
        doff = 0
        for idx, c in enumerate(CHUNKS):
            cc = c + (WRC if idx == 0 else 0)
            xt = xpool.tile([128, cc * B], f8, tag=f"xt{idx}", bufs=1)
            nc.sync.dma_start(xt[:, :], xt_d[:, doff * B : (doff + cc) * B])
            xts.append(xt)
            doff += cc
        wr8 = xts[0][:, CHUNKS[0] * B : (CHUNKS[0] + WRC) * B]
        # f32 consts row rides SWDGE (GPSIMD desc-gen): lands mid-stream,
        # well before tile 0's matmuls need the group-opening bias matmul
        wb = consts.tile([1, B + D + 1], f32)
        nc.gpsimd.dma_start(wb[:, :], wb_d[:, :])

        onesb = wb[0:1, 0:B]
        bres = wb[0:1, B : B + D + 1]
        # explicit zero-bias AP: a float bias would be lowered to a const AP
        # (skipped above), so activations get this tile instead
        zb = consts.tile([B, 1], f32)
        nc.vector.memset(zb[:, :], 0.0)
        # Sqrt bias -0.01/256: rt = sqrt((den - 1) * 0.01/256) = 0.1*|m|/16
        nb = consts.tile([B, 1], f32)
        nc.vector.memset(nb[:, :], -0.01 / (WS * WS))
        # DVE warm-up + early ACT Sqrt: pins the sqrt_and_others table
        # (holds Sqrt, Square, Copy) at t~0 instead of in the tail.
        scr = consts.tile([1, 1], f32)
        nc.vector.memset(scr[:, :], 1.0)
        scr2 = consts.tile([1, 1], f32)
        nc.scalar.activation(scr2[:, :], scr[:, :], AF.Sqrt, bias=zb[0:1, :])

        # S'[b, d] = WS*brow[d] + WS * sum_{p, t'} x[p,t',b] * W[p//4, d, t'%8]
        # The K=1 f32 bias matmul OPENS the [64, 17] group (start=True zeroes
        # the pad column too) — its wb const lands mid-stream via SWDGE well
        # before tile 0's matmuls; col 16 gets the constant 160 so the Square
        # accumulator below directly yields den = 1 + nsq (no DVE add).
        ps = psum.tile([B, D + 1], f32)
        nc.tensor.matmul(
            ps[:, :], onesb, bres, start=True, stop=False, skip_group_check=True
        )
        off = 0
        for t, c in enumerate(CHUNKS):
            xv = xts[t][:, : c * B].rearrange("p (c b) -> p c b", c=c)
            for j in range(c):
                i = (off + j) % I8
                nc.tensor.matmul(
                    ps[:, 0:D],
                    xv[:, j, :],
                    wr8[:, i * D : (i + 1) * D],
                    start=False,
                    stop=(off + j == TP - 1),
                    skip_group_check=True,
                )
            off += c

        # squash with m = S/10 = S'/(10*WS) folded into the scales:
        #   den = 1 + |m|^2 = sum_d (S'_pad/160)^2   (pad col -> the 1)
        #   rt = 0.1*|m|/16 = sqrt(den*0.01/256 - 0.01/256),  rec = 1/den
        #   v_row = S' * rt * rec
        sq = psum.tile([B, D + 1], f32)
        den = small.tile([B, 1], f32)
        nc.scalar.activation(
            sq[:, :], ps[:, :], AF.Square, bias=zb[:, :], scale=1.0 / (10.0 * WS),
            accum_out=den[:, :],
        )
        rt = small.tile([B, 1], f32)
        nc.scalar.activation(
            rt[:, :], den[:, :], AF.Sqrt, bias=nb[:, :], scale=0.01 / (WS * WS)
        )
        # rec on DVE overlaps the ACT Sqrt
        rec = small.tile([B, 1], f32)
        nc.vector.reciprocal(rec[:, :], den[:, :])

        # v_row = (S' * rt) * rec in one dual-scalar DVE op
        vrow = small.tile([B, D], f32)
        nc.vector.tensor_scalar(
            vrow[:, :],
            ps[:, 0:D],
            rt[:, :],
            rec[:, :],
            op0=mybir.AluOpType.mult,
            op1=mybir.AluOpType.mult,
        )
        nc.sync.dma_start(v_d[:, :], vrow[:, :])

    nc.finalize()

    # Teardown reorder: Tile emits the SP exit waits as five 2-wait
    # EventSemaphores draining every DMA lane + engine sem.  The out-DMA's
    # completion wait lands in the FIRST of them, so the other four decode
    # (~50 ns each) after the final DMA sem fires — pure added tail.  Move
    # that wait into the LAST instruction of the run (the wait set gating
    # the drains is unchanged, waits commute).
    insts = [i for blk in nc.m.functions[0].blocks for i in blk.instructions]

    # Disarm EventSemaphore waits that only reference their OWN engine's
    # tick semaphore: the engine executes its queue in order, so the
    # dependency already holds; the wait just parks the sequencer and
    # pushes the consumer's decode past the producing op's completion.
    eng_sem = {
        "EngineType.DVE": "DVE_",
        "EngineType.Activation": "Activation_",
        "EngineType.PE": "PE_",
        "EngineType.Pool": "Pool_",
        "EngineType.SP": "SP_",
    }
    for x in insts:
        if type(x).__name__ != "InstEventSemaphore" or x.sync_info is None:
            continue
        pref = eng_sem.get(str(x.engine))
        if not pref:
            continue
        for w in x.sync_info.on_wait:
            nm = w.ant_name or ""
            if nm.startswith(pref) and nm[len(pref):].isdigit():
                w.wait_value = 0

    dmas = [x for x in insts if type(x).__name__ == "InstDMACopy"]
    out_sem = dmas[-1].sync_info.on_update[0].id
    run = [
        x
        for x in insts
        if type(x).__name__ == "InstEventSemaphore"
        and str(x.engine) == "EngineType.SP"
        and x.sync_info is not None
        and len(x.sync_info.on_wait) == 2
        and any("DMAHW" in (w.ant_name or "") for w in x.sync_info.on_wait)
    ]
    if len(run) >= 2:
        src = None
        for x in run[:-1]:
            for w in x.sync_info.on_wait:
                if w.id == out_sem:
                    src = w
        if src is not None:
            dst = next(
                w for w in run[-1].sync_info.on_wait if w.id != out_sem
            )
            for fld in ("id", "ant_name", "wait_value"):
                a, b = getattr(src, fld), getattr(dst, fld)
                try:
                    setattr(src, fld, b)
                    setattr(dst, fld, a)
                except Exception:
                    pass
    return nc


def _host_inputs(x, W, Wb):
    import ml_dtypes

    f8 = ml_dtypes.float8_e3m4
    x = np.ascontiguousarray(np.asarray(x, dtype=np.float32)).reshape(BS, F)
    W = np.asarray(W, dtype=np.float32)
    Wb = np.asarray(Wb, dtype=np.float32)

    # wr8[p, i*16 + d] = WS * W[p//4, d, i]
    s_of_p = np.arange(128) // 4
    wrj = np.empty((128, I8, D), dtype=np.float32)
    for i in range(I8):
        wrj[:, i, :] = W[s_of_p, :, i]
    wr8 = (WS * wrj.reshape(128, I8 * D)).astype(f8)

    # x shards: [128 part, TP t', B b] fp8, flat f = part*TP + t'
    # (f = c*144 + p; partition part spans f in [part*288, part*288+288),
    #  always inside weight group part//4 since 1152 = 4*288), with the
    # wr8 columns spliced in right after tile 0's chunks
    c0 = CHUNKS[0] * B
    xs = []
    for k in range(NC):
        xk = np.ascontiguousarray(x[k * B : (k + 1) * B].T.astype(f8)).reshape(
            128, TP * B
        )
        xs.append(np.concatenate([xk[:, :c0], wr8, xk[:, c0:]], axis=1))

    # f32 consts row: ones for the K=1 bias matmul, WS*144*sum_s Wb, and
    # the 160 pad value whose square (after the 1/160 Square scale) is the
    # "+1" of the squash denominator
    wb = np.empty((1, B + D + 1), dtype=np.float32)
    wb[0, :B] = 1.0
    wb[0, B : B + D] = WS * HW * Wb.sum(axis=0)
    wb[0, B + D] = 10.0 * WS
    return xs, wb


def _run(x, W, Wb, trace=False):
    from concourse.bass_utils import run_bass_kernel_spmd

    xs, wb = _host_inputs(x, W, Wb)
    nc = _build_nc()
    in_maps = [{"xt": xs[k], "wb": wb} for k in range(NC)]
    res = run_bass_kernel_spmd(nc, in_maps, list(range(NC)), trace=trace)
    rows = np.concatenate([res.results[k]["v"] for k in range(NC)], axis=0)
    # unshard: replicate the (identical) caps into the full [BS, NO, D] shape
    out = np.ascontiguousarray(
        np.broadcast_to(rows.reshape(BS, 1, D), (BS, NO, D)), dtype=np.float32
    )
    return out, res


def _numpy_fallback(x, W, Wb, b0):
    """Generic routing on the host — only used if b0 is ever nonzero
    (the spec fills b0 with zeros, which collapses the routing; see top)."""
    x = np.asarray(x, np.float32)
    W = np.asarray(W, np.float32)
    Wb = np.asarray(Wb, np.float32)
    b0 = np.asarray(b0, np.float32)
    u = x.reshape(BS, 32, HW, I8)
    pred = np.einsum("bsni,soi->bsno", u, W) + Wb[None, :, None, :]
    pred = pred.reshape(BS, 32 * HW, D)
    b = np.broadcast_to(b0, (BS,) + b0.shape).copy()
    v = None
    for _ in range(3):
        e = np.exp(b - b.max(axis=1, keepdims=True))
        c = e / e.sum(axis=1, keepdims=True)
        s = np.einsum("boi,bid->bod", c, pred)
        nrm = np.linalg.norm(s, axis=2)
        coeff = (nrm * nrm / (1.0 + nrm * nrm)) / nrm
        v = s * coeff[:, :, None]
        b = b + np.einsum("bid,bod->boi", pred, v)
    return v.astype(np.float32)


def kernel(x, W, Wb, b0=None, **_ignored):
    if b0 is not None and np.any(np.asarray(b0)):
        return _numpy_fallback(x, W, Wb, b0)
    try:
        out, _ = _run(x, W, Wb, trace=False)
    except Exception:
        # one retry: the axon-tunneled device occasionally reports a
        # transient NRT_EXEC_UNIT_UNRECOVERABLE on first touch
        out, _ = _run(x, W, Wb, trace=False)
    return out


def kernel_traced(x, W, Wb, b0=None):
    """Like kernel() but also returns the BassKernelResults (exec_time_ns)."""
    return _run(x, W, Wb, trace=True)
